# revision 1
# baseline (speedup 1.0000x reference)
"""L2-distance attention layer on 8 Trainium2 NeuronCores.

Sharding: data-parallel over batch B=8 (one batch sample per core);
weights replicated. BatchNorm statistics (global over B and N) are
combined with an on-device AllReduce.

Math notes exploited:
  - The L2 distance matrix is symmetric with exactly-zero diagonal, so
    softmax(-l2) needs no row-max subtraction (row max is always 0) and
    exp tiles can be produced in [key, query] orientation directly.
  - d2 is computed in ONE matmul per tile via augmented vectors:
    [q; sq; 1]^T [-2q; 1; sq] -> sq_j - 2 q_j.q_i + sq_i.
  - conv biases bv, bt cancel exactly: attention rows sum to 1, so bv
    shifts t by a per-channel constant; constants cancel inside
    BatchNorm (train mode). They are dropped.
  - rstd = exp(-0.5*ln(var+eps)) so the tail reuses the exp table set
    instead of loading the sqrt/rsqrt tables.
"""
import sys
sys.path.insert(0, '/opt/trn_rl_repo')
import numpy as np

B, C, N = 8, 256, 2048
C4 = C // 4
P = 128
JC = N // P          # 16 j-chunks
NB = N // 512        # 4 i-blocks
NCORES = 8
BN_EPS = 1e-5
INV_BN = 1.0 / (B * N)

_CACHE = {}


def _build(sim=False):
    import concourse.bass as bass
    import concourse.tile as tile
    from concourse import bacc, mybir
    f32 = mybir.dt.float32

    nc = bacc.Bacc("TRN2", target_bir_lowering=False, debug=False,
                   num_devices=(1 if sim else NCORES))
    x_d = nc.dram_tensor("x", [P, 2, N], f32, kind="ExternalInput")
    wq_d = nc.dram_tensor("wqT", [P, 2, C4], f32, kind="ExternalInput")
    wv_d = nc.dram_tensor("wvT", [P, 2, C], f32, kind="ExternalInput")
    wt_d = nc.dram_tensor("wtT", [P, 2, C], f32, kind="ExternalInput")
    eye_d = nc.dram_tensor("eyem", [P, P], mybir.dt.uint8, kind="ExternalInput")
    gb_d = nc.dram_tensor("gb", [P, 2, 2], f32, kind="ExternalInput")
    out_d = nc.dram_tensor("out", [P, 2, N], f32, kind="ExternalOutput")

    AF = mybir.ActivationFunctionType
    OP = mybir.AluOpType

    with tile.TileContext(nc) as tc:
        with tc.tile_pool(name="perm", bufs=1) as perm, \
             tc.tile_pool(name="big", bufs=1) as bigp, \
             tc.tile_pool(name="dram", bufs=1, space="DRAM") as dram:
            # ---- permanent small tiles
            xw = perm.tile([P, 2, N], f32)
            nc.sync.dma_start(xw[:], x_d.ap())
            wq = perm.tile([P, 2, C4], f32)
            nc.sync.dma_start(wq[:], wq_d.ap())
            wv = perm.tile([P, 2, C], f32)
            nc.sync.dma_start(wv[:], wv_d.ap())
            wt = perm.tile([P, 2, C], f32)
            nc.sync.dma_start(wt[:], wt_d.ap())
            eye = perm.tile([P, P], mybir.dt.uint8)
            nc.sync.dma_start(eye[:], eye_d.ap())
            gb = perm.tile([P, 2, 2], f32)
            nc.sync.dma_start(gb[:], gb_d.ap())
            zer = perm.tile([P, P], f32)
            nc.vector.memset(zer[:], 0.0)
            ones64 = perm.tile([C4, 1], f32)
            nc.vector.memset(ones64[:], 1.0)
            vT = perm.tile([P, JC, C], f32)
            dencol = perm.tile([P, JC], f32)
            rep = perm.tile([P, N], f32)
            l2big = bigp.tile([P, JC, N], f32)   # 8 KB/part * 16 = 128 KB/part
            xr = perm.tile([P, 2, N], f32)
            stat = perm.tile([P, 8], f32)

            # ---- setup: q, sq, A/B bases, vT
            _ABpool = tc.tile_pool(name="ab", bufs=1)
            abp = _ABpool.__enter__()
            _AB = (abp.tile([P, N], f32, tag="A", name="At"),
                   abp.tile([P, N], f32, tag="B", name="Bt"))
            with tc.tile_pool(name="ps_set", bufs=2, space="PSUM") as pss:
                At, Bt = _AB
                nc.vector.memset(At[:], 0.0)
                nc.vector.memset(Bt[:], 0.0)
                for nb in range(NB):
                    pq = pss.tile([C4, 512], f32, tag="pq")
                    nc.tensor.matmul(pq[:], lhsT=wq[:, 0, :],
                                     rhs=xw[:, 0, nb * 512:(nb + 1) * 512],
                                     start=True, stop=False)
                    nc.tensor.matmul(pq[:], lhsT=wq[:, 1, :],
                                     rhs=xw[:, 1, nb * 512:(nb + 1) * 512],
                                     start=False, stop=True)
                    nc.vector.tensor_copy(out=At[0:C4, nb * 512:(nb + 1) * 512],
                                          in_=pq[:])
                # q^2 into B rows 0:64 (scratch), then sq row
                nc.vector.tensor_tensor(out=Bt[0:C4, :], in0=At[0:C4, :],
                                        in1=At[0:C4, :], op=OP.mult)
                for nb in range(NB):
                    psq = pss.tile([1, 512], f32, tag="psq")
                    nc.tensor.matmul(psq[:],
                                     lhsT=ones64[:], rhs=Bt[0:C4, nb * 512:(nb + 1) * 512],
                                     start=True, stop=True)
                    nc.vector.tensor_copy(out=At[C4:C4 + 1, nb * 512:(nb + 1) * 512], in_=psq[:])
                    nc.vector.tensor_copy(out=Bt[96:97, nb * 512:(nb + 1) * 512], in_=psq[:])
                # overwrite B rows 0:64 with -2q (after sq matmuls read them)
                nc.vector.tensor_scalar(out=Bt[0:C4, :], in0=At[0:C4, :],
                                        scalar1=-2.0, scalar2=0.0,
                                        op0=OP.mult, op1=OP.add)
                nc.vector.memset(At[96:97, :], 1.0)
                nc.vector.memset(Bt[C4:C4 + 1, :], 1.0)
                # vT
                for jc in range(JC):
                    pv = pss.tile([P, C], f32, tag="pv")
                    nc.tensor.matmul(pv[:], lhsT=xw[:, 0, jc * P:(jc + 1) * P],
                                     rhs=wv[:, 0, :], start=True, stop=False)
                    nc.tensor.matmul(pv[:], lhsT=xw[:, 1, jc * P:(jc + 1) * P],
                                     rhs=wv[:, 1, :], start=False, stop=True)
                    nc.vector.tensor_copy(out=vT[:, jc, :], in_=pv[:])

            # ---- phase A: d2 tiles -> sqrt -> l2big  (ps_set closed)
            with tc.tile_pool(name="abx", bufs=1) as abp2:
                At, Bt = _AB[0], _AB[1]
                with tc.tile_pool(name="ps_d2", bufs=2, space="PSUM") as psd:
                    for a in range(JC):
                        pd2 = psd.tile([P, N], f32, tag="d2")
                        for nb in range(NB):
                            nc.tensor.matmul(pd2[:, nb * 512:(nb + 1) * 512],
                                             lhsT=At[:, a * P:(a + 1) * P],
                                             rhs=Bt[:, nb * 512:(nb + 1) * 512],
                                             start=True, stop=True)
                        nc.scalar.activation(l2big[:, a, :], pd2[:], AF.Sqrt)
                        # exact-zero the diagonal block (kills NaN from sqrt(neg))
                        nc.vector.copy_predicated(
                            out=l2big[:, a, a * P:(a + 1) * P],
                            mask=eye[:], data=zer[:])

            _ABpool.__exit__(None, None, None)
            # x2 reload (reuses A/B space after pool close; overlaps phase B)
            if True:
                x2 = xw
                # ---- phase B: exp (+den accum) and attn@v
                psav_cm = tc.tile_pool(name="ps_av", bufs=1, space="PSUM")
                psav = psav_cm.__enter__()
                pav = [psav.tile([P, 512], f32, tag=f"av{i}", name=f"pav{i}") for i in range(8)]
                for a in range(JC):
                    Pst = l2big[:, a, :]
                    nc.scalar.activation(Pst, l2big[:, a, :], AF.Exp,
                                         scale=-1.0,
                                         accum_out=dencol[:, a:a + 1])
                    for oc in range(2):
                        for ib in range(NB):
                            nc.tensor.matmul(
                                pav[oc * NB + ib][:],
                                lhsT=vT[:, a, oc * P:(oc + 1) * P],
                                rhs=Pst[:, ib * 512:(ib + 1) * 512],
                                start=(a == 0), stop=(a == JC - 1))

                # ---- denominators -> reciprocal -> broadcast row
                rden = perm.tile([P, JC], f32)
                nc.vector.reciprocal(rden[:], dencol[:])
                dden = dram.tile([N], f32)
                nc.sync.dma_start(dden.rearrange("(a r) -> r a", r=P), rden[:])
                bsrc = bass.AP(tensor=dden.tensor, offset=dden.offset,
                               ap=[[0, P], [1, N]])
                nc.sync.dma_start(rep[:], bsrc)

                # ---- x_r = pav * rep (normalize)
                for oc in range(2):
                    for ib in range(NB):
                        nc.vector.tensor_tensor(
                            out=xr[:, oc, ib * 512:(ib + 1) * 512],
                            in0=pav[oc * NB + ib][:],
                            in1=rep[:, ib * 512:(ib + 1) * 512], op=OP.mult)

                psav_cm.__exit__(None, None, None)
                # ---- t = wtT . xr (write back into xr in place per block)
                with tc.tile_pool(name="ps_t", bufs=2, space="PSUM") as pst:
                    s1p = [[perm.tile([P, 1], f32, name=f"s1_{o}_{n}", tag=f"s1_{o}_{n}")
                            for n in range(NB)] for o in range(2)]
                    for nb in range(NB):
                        ptl = []
                        for oc2 in range(2):
                            pt = pst.tile([P, 512], f32, tag=f"t{oc2}", name=f"pt{oc2}")
                            nc.tensor.matmul(pt[:], lhsT=wt[:, 0, oc2 * P:(oc2 + 1) * P],
                                             rhs=xr[:, 0, nb * 512:(nb + 1) * 512],
                                             start=True, stop=False)
                            nc.tensor.matmul(pt[:], lhsT=wt[:, 1, oc2 * P:(oc2 + 1) * P],
                                             rhs=xr[:, 1, nb * 512:(nb + 1) * 512],
                                             start=False, stop=True)
                            ptl.append(pt)
                        for oc2 in range(2):
                            nc.vector.tensor_scalar(
                                out=xr[:, oc2, nb * 512:(nb + 1) * 512],
                                in0=ptl[oc2][:], scalar1=1.0, scalar2=0.0,
                                op0=OP.mult, op1=OP.add,
                                accum_out=s1p[oc2][nb][:])

                # ---- stats: s1 = sum(t), s2 = sum(t^2)
                for oc2 in range(2):
                    nc.vector.tensor_tensor(out=stat[:, oc2:oc2 + 1],
                                            in0=s1p[oc2][0][:], in1=s1p[oc2][1][:],
                                            op=OP.add)
                    nc.vector.tensor_tensor(out=stat[:, oc2:oc2 + 1],
                                            in0=stat[:, oc2:oc2 + 1], in1=s1p[oc2][2][:],
                                            op=OP.add)
                    nc.vector.tensor_tensor(out=stat[:, oc2:oc2 + 1],
                                            in0=stat[:, oc2:oc2 + 1], in1=s1p[oc2][3][:],
                                            op=OP.add)
                    nc.vector.scalar_tensor_tensor(
                        out=l2big[:, oc2, :], in0=xr[:, oc2, :], scalar=1.0,
                        in1=xr[:, oc2, :], op0=OP.mult, op1=OP.mult,
                        accum_out=stat[:, 2 + oc2:3 + oc2])

                # ---- AllReduce stats across 8 cores
                cin = dram.tile([P, 4], f32)
                cout = dram.tile([P, 4], f32, addr_space="Shared")
                nc.sync.dma_start(cin[:], stat[:, 0:4])
                if sim:
                    nc.sync.dma_start(cout[:], cin[:])
                else:
                    nc.gpsimd.collective_compute(
                        "AllReduce", OP.add,
                        replica_groups=[list(range(NCORES))],
                        ins=[cin.opt()], outs=[cout.opt()])
                sg = perm.tile([P, 4], f32)
                nc.sync.dma_start(sg[:], cout[:])

                # ---- BN affine params per chunk
                epst = perm.tile([P, 1], f32)
                nc.vector.memset(epst[:], BN_EPS)
                Ak = [perm.tile([P, 1], f32, name=f"Ak{o}", tag=f"Ak{o}") for o in range(2)]
                Bk = [perm.tile([P, 1], f32, name=f"Bk{o}", tag=f"Bk{o}") for o in range(2)]
                mean = perm.tile([P, 2], f32)
                var = perm.tile([P, 2], f32)
                for oc2 in range(2):
                    nc.vector.tensor_scalar(out=mean[:, oc2:oc2 + 1],
                                            in0=sg[:, oc2:oc2 + 1],
                                            scalar1=INV_BN, scalar2=0.0,
                                            op0=OP.mult, op1=OP.add)
                    # var = s2/BN - mean^2
                    nc.vector.tensor_scalar(out=var[:, oc2:oc2 + 1],
                                            in0=sg[:, 2 + oc2:3 + oc2],
                                            scalar1=INV_BN, scalar2=0.0,
                                            op0=OP.mult, op1=OP.add)
                    nc.vector.scalar_tensor_tensor(
                        out=var[:, oc2:oc2 + 1], in0=mean[:, oc2:oc2 + 1],
                        scalar=-1.0, in1=mean[:, oc2:oc2 + 1],
                        op0=OP.mult, op1=OP.mult)
                    nc.vector.tensor_scalar(out=var[:, oc2:oc2 + 1],
                                            in0=var[:, oc2:oc2 + 1],
                                            scalar1=-1.0, scalar2=0.0,
                                            op0=OP.mult, op1=OP.add)
                    nc.vector.scalar_tensor_tensor(
                        out=var[:, oc2:oc2 + 1], in0=sg[:, 2 + oc2:3 + oc2],
                        scalar=INV_BN, in1=var[:, oc2:oc2 + 1],
                        op0=OP.mult, op1=OP.subtract)
                    # rstd = exp(-0.5 ln(var+eps))
                    nc.scalar.activation(var[:, oc2:oc2 + 1], var[:, oc2:oc2 + 1],
                                         AF.Ln, bias=epst[:])
                    nc.scalar.activation(var[:, oc2:oc2 + 1], var[:, oc2:oc2 + 1],
                                         AF.Exp, scale=-0.5)
                    # Ak = gamma*rstd ; Bk = beta - mean*Ak
                    nc.vector.tensor_tensor(out=Ak[oc2][:], in0=gb[:, oc2, 0:1],
                                            in1=var[:, oc2:oc2 + 1], op=OP.mult)
                    nc.vector.tensor_tensor(out=Bk[oc2][:], in0=mean[:, oc2:oc2 + 1],
                                            in1=Ak[oc2][:], op=OP.mult)
                    nc.vector.tensor_tensor(out=Bk[oc2][:], in0=gb[:, oc2, 1:2],
                                            in1=Bk[oc2][:], op=OP.subtract)

                # ---- out = x + relu(Ak*t + Bk)
                for oc2 in range(2):
                    u = l2big[:, 4 + oc2, :]
                    nc.scalar.activation(u, xr[:, oc2, :], AF.Relu,
                                         scale=Ak[oc2][:], bias=Bk[oc2][:])
                    o = l2big[:, 6 + oc2, :]
                    nc.vector.tensor_tensor(out=o, in0=x2[:, oc2, :], in1=u,
                                            op=OP.add)
                    nc.sync.dma_start(out_d.ap()[:, oc2, :], o)

    nc.compile()
    return nc


def _get_nc():
    if "nc" not in _CACHE:
        _CACHE["nc"] = _build()
    return _CACHE["nc"]


def kernel(x, wq, wv, bv, wt, bt, gamma, beta):
    from concourse.bass_utils import run_bass_kernel_spmd

    x = np.asarray(x, dtype=np.float32)
    # host-side weight re-layouts (tiny)
    wqT = np.ascontiguousarray(
        np.asarray(wq, np.float32).T.reshape(2, P, C4).transpose(1, 0, 2))
    wvT = np.ascontiguousarray(
        np.asarray(wv, np.float32).T.reshape(2, P, C).transpose(1, 0, 2))
    wtT = np.ascontiguousarray(
        np.asarray(wt, np.float32).T.reshape(2, P, C).transpose(1, 0, 2))
    eyem = np.eye(P, dtype=np.uint8)
    gbh = np.stack([np.asarray(gamma, np.float32).reshape(2, P).T,
                    np.asarray(beta, np.float32).reshape(2, P).T],
                   axis=2).astype(np.float32)  # [P, 2, 2]
    gbh = np.ascontiguousarray(gbh)

    nc = _get_nc()
    in_maps = []
    for b in range(NCORES):
        xb = np.ascontiguousarray(
            x[b].reshape(2, P, N).transpose(1, 0, 2))  # [P, 2, N]
        in_maps.append({"x": xb, "wqT": wqT, "wvT": wvT,
                        "wtT": wtT, "eyem": eyem, "gb": gbh})
    _CACHE["last_in_maps"] = in_maps
    res = run_bass_kernel_spmd(nc, in_maps, core_ids=list(range(NCORES)))
    _CACHE["last_res"] = res
    out = np.empty((B, C, N), dtype=np.float32)
    for b in range(NCORES):
        ob = res.results[b]["out"]  # [P, 2, N]
        out[b] = ob.transpose(1, 0, 2).reshape(C, N)
    return out



# revision 17
# speedup vs baseline: 1.0528x; 1.0528x over previous
"""L2-distance attention layer on 8 Trainium2 NeuronCores.

Sharding: data-parallel over batch B=8 (one batch sample per core);
weights replicated. BatchNorm statistics (global over B and N) are
combined with an on-device AllReduce.

Math notes exploited:
  - The L2 distance matrix is symmetric with exactly-zero diagonal, so
    softmax(-l2) needs no row-max subtraction (row max is always 0).
  - d2 is computed in ONE matmul per tile via augmented vectors:
    [q; sq; 1]^T [-2q; 1; sq] -> sq_j - 2 q_j.q_i + sq_i.
  - conv biases bv, bt cancel exactly: attention rows sum to 1, so bv
    shifts t by a per-channel constant; constants cancel inside
    BatchNorm (train mode). They are dropped.
  - The softmax normalization commutes with the channel matmul:
    t = wt@(xr_un * rep) = (wt@xr_un) * rep, so the reciprocal-денom
    broadcast is folded into the post-matmul PSUM->SBUF move, off the
    critical path.
  - rstd = exp(-0.5*ln(var+eps)) so the tail reuses the exp table set
    instead of loading the sqrt/rsqrt tables.

Performance structure (vs the all-fp32 baseline):
  - All dense matmuls run at 1 cycle/column: float32r for the
    setup/d2/t GEMMs (tf32-class accuracy), float16 for attn@v.
  - sqrt and exp live in different ACT table sets, so the kernel is
    split into phase A (all 16 sqrt chunks) and phase B (all 16 exp
    chunks) with exactly one table load each; the exp^T@v matmuls
    chase the exps chunk-by-chunk.
  - l2/E matrix is stored once in SBUF as fp16 [P, 16, 2048]
    (64 KB/partition); exp runs in-place with an fp32 row-sum
    accumulator for the softmax denominators.
  - Setup is a 4-block pipeline (x DMA -> q matmul -> copies/squares
    split between ACT and DVE -> sq matmul), with the vT matmuls
    interleaved, so the first sqrt starts ~15us in.
"""
import sys
sys.path.insert(0, '/opt/trn_rl_repo')
import numpy as np

B, C, N = 8, 256, 2048
C4 = C // 4
P = 128
JC = N // P          # 16 j-chunks
NB = N // 512        # 4 column blocks of 512
NCORES = 8
BN_EPS = 1e-5
INV_BN = 1.0 / (B * N)

_CACHE = {}


def _build(sim=False):
    import concourse.bass as bass
    import concourse.tile as tile
    from concourse import bacc, mybir
    f32 = mybir.dt.float32
    f32r = mybir.dt.float32r
    f16 = mybir.dt.float16

    nc = bacc.Bacc("TRN2", target_bir_lowering=False, debug=False,
                   num_devices=(1 if sim else NCORES))
    x_d = nc.dram_tensor("x", [2, P, N], f32r, kind="ExternalInput")
    wq_d = nc.dram_tensor("wqT", [P, 2, C4], f32r, kind="ExternalInput")
    wv_d = nc.dram_tensor("wvT", [P, 2, C], f32r, kind="ExternalInput")
    wt_d = nc.dram_tensor("wtT", [P, 2, C], f32r, kind="ExternalInput")
    eye_d = nc.dram_tensor("eyem", [P, P], mybir.dt.uint8, kind="ExternalInput")
    gb_d = nc.dram_tensor("gb", [P, 2, 2], f32, kind="ExternalInput")
    one_d = nc.dram_tensor("onesrow", [1, N], f32r, kind="ExternalInput")
    out_d = nc.dram_tensor("out", [2, P, N], f32r, kind="ExternalOutput")

    AF = mybir.ActivationFunctionType
    OP = mybir.AluOpType

    def r(ap):
        return ap if ap.dtype == f32r else ap.bitcast(f32r)

    def blk(nb):
        return slice(512 * nb, 512 * (nb + 1))

    with tile.TileContext(nc) as tc:
        with tc.tile_pool(name="perm", bufs=1) as perm, \
             tc.tile_pool(name="dram", bufs=1, space="DRAM") as dram:
            # ---- permanent SBUF tiles
            xw = perm.tile([P, 2, N], f32r)
            wq = perm.tile([P, 2, C4], f32r)
            wv = perm.tile([P, 2, C], f32r)
            wt = perm.tile([P, 2, C], f32r)
            eye = perm.tile([P, P], mybir.dt.uint8)
            gb = perm.tile([P, 2, 2], f32)
            zerh = perm.tile([P, P], f16)
            ones64 = perm.tile([C4, 1], f32r)
            At = perm.tile([97, N], f32r)   # rows: 0-63 q, 64 sq, 96 ones
            Bt = perm.tile([97, N], f32r)   # rows: 0-63 -2q, 64 ones, 96 sq
            vT = perm.tile([P, JC, C], f16)
            dencol = perm.tile([P, JC], f32)
            rep = perm.tile([P, N], f32)
            ebig = perm.tile([P, JC, N], f16)   # l2 then E, in place
            xr = perm.tile([P, 2, N], f32r)
            stat = perm.tile([P, 8], f32)

            # zero the dead augmentation rows 64-95 before the ones rows
            # land (rows 64/96 are rewritten below; DVE is idle this early)
            nc.vector.memset(At[64:96, :].bitcast(f32), 0.0)
            nc.vector.memset(Bt[64:96, :].bitcast(f32), 0.0)
            # DMA order = need order: tiny constants and x block 0 first,
            # wq next, the fat/late tensors at the back.
            nc.sync.dma_start(At[96:97, :], one_d.ap())
            nc.sync.dma_start(Bt[64:65, :], one_d.ap())
            for o in range(2):
                nc.sync.dma_start(xw[:, o, blk(0)], x_d.ap()[o, :, blk(0)])
            nc.sync.dma_start(wq[:], wq_d.ap())
            for nb in range(1, NB):
                for o in range(2):
                    nc.sync.dma_start(xw[:, o, blk(nb)], x_d.ap()[o, :, blk(nb)])
            nc.sync.dma_start(wv[:], wv_d.ap())
            nc.sync.dma_start(eye[:], eye_d.ap())
            nc.sync.dma_start(wt[:], wt_d.ap())
            nc.sync.dma_start(gb[:], gb_d.ap())

            nc.vector.memset(ones64[:].bitcast(f32), 1.0)
            # force the initial ACT table load to be the sqrt set (which
            # also contains square/copy/relu) before any other activation
            nc.scalar.activation(stat[0:C4, 7:8], ones64[:], AF.Sqrt)
            nc.vector.memset(zerh[:], 0.0)

            # ---- setup pipeline over the 4 column blocks:
            #   q mm -> At copy (DVE) -> q^2 (ACT Square, scratch in xr)
            #   -> sq mm -> At64 (ACT) / Bt96 (DVE); -2q (DVE) hangs off
            #   the At copy only. The vT matmuls come after, overlapping
            #   phase A via their own small PSUM pool.
            with tc.tile_pool(name="ps_set", bufs=2, space="PSUM") as pss:
                for nb in range(NB):
                    pq = pss.tile([C4, 512], f32, tag="pq")
                    nc.tensor.matmul(pq[:], lhsT=r(wq[:, 0, :]),
                                     rhs=r(xw[:, 0, blk(nb)]),
                                     start=True, stop=False)
                    nc.tensor.matmul(pq[:], lhsT=r(wq[:, 1, :]),
                                     rhs=r(xw[:, 1, blk(nb)]),
                                     start=False, stop=True)
                    nc.vector.tensor_copy(out=At[0:C4, blk(nb)], in_=pq[:])
                    # q^2 scratch in xr (dead until the tail)
                    nc.scalar.activation(xr[0:C4, 0, blk(nb)],
                                         At[0:C4, blk(nb)], AF.Square)
                    psq = pss.tile([1, 512], f32, tag="psq")
                    nc.tensor.matmul(psq[:], lhsT=r(ones64[:]),
                                     rhs=r(xr[0:C4, 0, blk(nb)]),
                                     start=True, stop=True)
                    nc.scalar.activation(At[C4:C4 + 1, blk(nb)], psq[:], AF.Copy)
                    nc.vector.tensor_copy(out=Bt[96:97, blk(nb)], in_=psq[:])
                    nc.vector.tensor_scalar(out=Bt[0:C4, blk(nb)],
                                            in0=At[0:C4, blk(nb)],
                                            scalar1=-2.0, scalar2=0.0,
                                            op0=OP.mult, op1=OP.add)

            # ---- phase A: d2 -> sqrt -> l2 (fp16) + diagonal zero.
            # pd2 is 3x [P, 1024] (6 banks) so the vT pool (2 banks) can
            # coexist and the vT matmuls/copies fill phase A's PE/DVE slack.
            with tc.tile_pool(name="ps_v", bufs=2, space="PSUM") as psv, \
                 tc.tile_pool(name="ps_d2", bufs=3, space="PSUM") as psd:
                for a in range(JC):
                    for h in range(2):
                        cols = slice(1024 * h, 1024 * (h + 1))
                        pd2 = psd.tile([P, 1024], f32, tag="d2")
                        for q2 in range(2):
                            nc.tensor.matmul(pd2[:, 512 * q2:512 * (q2 + 1)],
                                             lhsT=r(At[:, a * P:(a + 1) * P]),
                                             rhs=r(Bt[:, 1024 * h + 512 * q2:
                                                      1024 * h + 512 * (q2 + 1)]),
                                             start=True, stop=True)
                        nc.scalar.activation(ebig[:, a, cols], pd2[:], AF.Sqrt)
                        if a // 8 == h:
                            # exact-zero the diagonal block (kills NaN
                            # from sqrt of tiny negatives)
                            nc.vector.copy_predicated(
                                out=ebig[:, a, a * P:(a + 1) * P],
                                mask=eye[:], data=zerh[:])
                for jc in range(JC):
                    pv = psv.tile([P, C], f32, tag="pv")
                    nc.tensor.matmul(pv[:], lhsT=r(xw[:, 0, jc * P:(jc + 1) * P]),
                                     rhs=r(wv[:, 0, :]), start=True, stop=False)
                    nc.tensor.matmul(pv[:], lhsT=r(xw[:, 1, jc * P:(jc + 1) * P]),
                                     rhs=r(wv[:, 1, :]), start=False, stop=True)
                    nc.vector.tensor_copy(out=vT[:, jc, :], in_=pv[:])

            # ---- phase B: exp (+den accum) chased by attn@v matmuls
            psav_cm = tc.tile_pool(name="ps_av", bufs=1, space="PSUM")
            psav = psav_cm.__enter__()
            pav = [psav.tile([P, 512], f32, tag=f"av{i}", name=f"pav{i}")
                   for i in range(8)]
            for a in range(JC):
                Pst = ebig[:, a, :]
                nc.scalar.activation(Pst, Pst, AF.Exp, scale=-1.0,
                                     accum_out=dencol[:, a:a + 1])
                for oc in range(2):
                    for ib in range(NB):
                        nc.tensor.matmul(
                            pav[oc * NB + ib][:],
                            lhsT=vT[:, a, oc * P:(oc + 1) * P],
                            rhs=Pst[:, ib * 512:(ib + 1) * 512],
                            start=(a == 0), stop=(a == JC - 1))
                if a % 4 == 3:
                    # denominators -> reciprocal -> broadcast row; four
                    # quarters (one per tail column block) so only the
                    # last quarter's round trip trails the final exp.
                    h = a // 4
                    rden = perm.tile([P, 4], f32, tag=f"rden{h}",
                                     name=f"rden{h}")
                    nc.vector.reciprocal(rden[:], dencol[:, 4 * h:4 * (h + 1)])
                    dden = dram.tile([512], f32, tag=f"dden{h}",
                                     name=f"dden{h}")
                    nc.sync.dma_start(dden.rearrange("(a r) -> r a", r=P), rden[:])
                    bsrc = bass.AP(tensor=dden.tensor, offset=dden.offset,
                                   ap=[[0, P], [1, 512]])
                    nc.sync.dma_start(rep[:, 512 * h:512 * (h + 1)], bsrc)

            # ---- xr_un = pav (move to SBUF), ib-major so t can chase;
            # oc=0 rides ACT (Copy is in every table set), oc=1 rides DVE.
            for ib in range(NB):
                nc.scalar.activation(xr[:, 0, blk(ib)], pav[ib][:], AF.Copy)
                nc.vector.tensor_copy(out=xr[:, 1, blk(ib)], in_=pav[NB + ib][:])

            psav_cm.__exit__(None, None, None)
            # ---- t = (wtT . xr_un) * rep, written back into xr in place;
            # the rep multiply carries the s1 accumulation.
            with tc.tile_pool(name="ps_t", bufs=2, space="PSUM") as pst:
                s1p = [[perm.tile([P, 1], f32, name=f"s1_{o}_{n}", tag=f"s1_{o}_{n}")
                        for n in range(NB)] for o in range(2)]
                s2p = [[perm.tile([P, 1], f32, name=f"s2_{o}_{n}", tag=f"s2_{o}_{n}")
                        for n in range(NB)] for o in range(2)]
                for nb in range(NB):
                    ptl = []
                    for oc2 in range(2):
                        pt = pst.tile([P, 512], f32, tag=f"t{oc2}", name=f"pt{oc2}")
                        nc.tensor.matmul(pt[:], lhsT=r(wt[:, 0, oc2 * P:(oc2 + 1) * P]),
                                         rhs=r(xr[:, 0, blk(nb)]),
                                         start=True, stop=False)
                        nc.tensor.matmul(pt[:], lhsT=r(wt[:, 1, oc2 * P:(oc2 + 1) * P]),
                                         rhs=r(xr[:, 1, blk(nb)]),
                                         start=False, stop=True)
                        ptl.append(pt)
                    for oc2 in range(2):
                        nc.vector.scalar_tensor_tensor(
                            out=xr[:, oc2, blk(nb)], in0=ptl[oc2][:],
                            scalar=1.0, in1=rep[:, blk(nb)],
                            op0=OP.mult, op1=OP.mult,
                            accum_out=s1p[oc2][nb][:])
                        # s2 partial (ACT Square, per block, chases the STT)
                        nc.scalar.activation(
                            out=ebig[:, oc2, blk(nb)],
                            in_=xr[:, oc2, blk(nb)], func=AF.Square,
                            accum_out=s2p[oc2][nb][:])

                # ---- stats: s1/s2 partial sums -> stat[:, 0:4]
                for oc2 in range(2):
                    for col, parts in ((oc2, s1p[oc2]), (2 + oc2, s2p[oc2])):
                        nc.vector.tensor_tensor(out=stat[:, col:col + 1],
                                                in0=parts[0][:], in1=parts[1][:],
                                                op=OP.add)
                        nc.vector.tensor_tensor(out=stat[:, col:col + 1],
                                                in0=stat[:, col:col + 1],
                                                in1=parts[2][:], op=OP.add)
                        nc.vector.tensor_tensor(out=stat[:, col:col + 1],
                                                in0=stat[:, col:col + 1],
                                                in1=parts[3][:], op=OP.add)

                # ---- AllReduce stats across 8 cores
                cin = dram.tile([P, 4], f32)
                cout = dram.tile([P, 4], f32, addr_space="Shared")
                nc.sync.dma_start(cin[:], stat[:, 0:4])
                if sim:
                    nc.sync.dma_start(cout[:], cin[:])
                else:
                    nc.gpsimd.collective_compute(
                        "AllReduce", OP.add,
                        replica_groups=[list(range(NCORES))],
                        ins=[cin.opt()], outs=[cout.opt()])
                sg = perm.tile([P, 4], f32)
                nc.sync.dma_start(sg[:], cout[:])

                # ---- BN affine params per channel half
                epst = perm.tile([P, 1], f32)
                nc.vector.memset(epst[:], BN_EPS)
                Ak = [perm.tile([P, 1], f32, name=f"Ak{o}", tag=f"Ak{o}") for o in range(2)]
                Bk = [perm.tile([P, 1], f32, name=f"Bk{o}", tag=f"Bk{o}") for o in range(2)]
                mean = perm.tile([P, 2], f32)
                var = perm.tile([P, 2], f32)
                for oc2 in range(2):
                    nc.vector.tensor_scalar(out=mean[:, oc2:oc2 + 1],
                                            in0=sg[:, oc2:oc2 + 1],
                                            scalar1=INV_BN, scalar2=0.0,
                                            op0=OP.mult, op1=OP.add)
                    # var = s2/BN - mean^2
                    nc.vector.scalar_tensor_tensor(
                        out=var[:, oc2:oc2 + 1], in0=mean[:, oc2:oc2 + 1],
                        scalar=1.0, in1=mean[:, oc2:oc2 + 1],
                        op0=OP.mult, op1=OP.mult)
                    nc.vector.scalar_tensor_tensor(
                        out=var[:, oc2:oc2 + 1], in0=sg[:, 2 + oc2:3 + oc2],
                        scalar=INV_BN, in1=var[:, oc2:oc2 + 1],
                        op0=OP.mult, op1=OP.subtract)
                    # rstd = exp(-0.5 ln(var+eps))
                    nc.scalar.activation(var[:, oc2:oc2 + 1], var[:, oc2:oc2 + 1],
                                         AF.Ln, bias=epst[:])
                    nc.scalar.activation(var[:, oc2:oc2 + 1], var[:, oc2:oc2 + 1],
                                         AF.Exp, scale=-0.5)
                    # Ak = gamma*rstd ; Bk = beta - mean*Ak
                    nc.vector.tensor_tensor(out=Ak[oc2][:], in0=gb[:, oc2, 0:1],
                                            in1=var[:, oc2:oc2 + 1], op=OP.mult)
                    nc.vector.tensor_tensor(out=Bk[oc2][:], in0=mean[:, oc2:oc2 + 1],
                                            in1=Ak[oc2][:], op=OP.mult)
                    nc.vector.tensor_tensor(out=Bk[oc2][:], in0=gb[:, oc2, 1:2],
                                            in1=Bk[oc2][:], op=OP.subtract)

                # ---- out = x + relu(Ak*t + Bk), 8 pipelined units;
                # the residual add runs in place over xw.
                for nb in range(NB):
                    for oc2 in range(2):
                        u = ebig[:, 4 + 4 * oc2 + nb, 0:1024].bitcast(f32r)
                        nc.scalar.activation(u, xr[:, oc2, blk(nb)], AF.Relu,
                                             scale=Ak[oc2][:], bias=Bk[oc2][:])
                        nc.vector.tensor_tensor(out=xw[:, oc2, blk(nb)],
                                                in0=xw[:, oc2, blk(nb)],
                                                in1=u, op=OP.add)
                        nc.sync.dma_start(out_d.ap()[oc2, :, blk(nb)],
                                          xw[:, oc2, blk(nb)])

    nc.compile()
    return nc


def _get_nc():
    if "nc" not in _CACHE:
        _CACHE["nc"] = _build()
    return _CACHE["nc"]


def kernel(x, wq, wv, bv, wt, bt, gamma, beta):
    from concourse.bass_utils import run_bass_kernel_spmd

    x = np.ascontiguousarray(np.asarray(x, dtype=np.float32))
    # host-side weight re-layouts (tiny)
    wqT = np.ascontiguousarray(
        np.asarray(wq, np.float32).T.reshape(2, P, C4).transpose(1, 0, 2))
    wvT = np.ascontiguousarray(
        np.asarray(wv, np.float32).T.reshape(2, P, C).transpose(1, 0, 2))
    wtT = np.ascontiguousarray(
        np.asarray(wt, np.float32).T.reshape(2, P, C).transpose(1, 0, 2))
    eyem = np.eye(P, dtype=np.uint8)
    gbh = np.stack([np.asarray(gamma, np.float32).reshape(2, P).T,
                    np.asarray(beta, np.float32).reshape(2, P).T],
                   axis=2).astype(np.float32)  # [P, 2, 2]
    gbh = np.ascontiguousarray(gbh)
    onesr = np.ones((1, N), dtype=np.float32)

    nc = _get_nc()
    in_maps = []
    for b in range(NCORES):
        xb = x[b].reshape(2, P, N)  # contiguous view, no copy
        in_maps.append({"x": xb, "wqT": wqT, "wvT": wvT,
                        "wtT": wtT, "eyem": eyem, "gb": gbh,
                        "onesrow": onesr})
    _CACHE["last_in_maps"] = in_maps
    res = run_bass_kernel_spmd(nc, in_maps, core_ids=list(range(NCORES)))
    _CACHE["last_res"] = res
    out = np.empty((B, C, N), dtype=np.float32)
    for b in range(NCORES):
        out[b] = res.results[b]["out"].reshape(C, N)
    return out


# revision 27
# speedup vs baseline: 1.1406x; 1.0835x over previous
"""L2-distance attention layer on 8 Trainium2 NeuronCores.

Sharding: data-parallel over batch B=8 (one batch sample per core);
weights replicated. BatchNorm statistics (global over B and N) are
combined with an on-device AllReduce.

Math notes exploited:
  - The L2 distance matrix is symmetric with exactly-zero diagonal, so
    softmax(-l2) needs no row-max subtraction (row max is always 0).
  - d2 is computed in ONE matmul per tile via augmented vectors:
    [q; sq; 1]^T [-2q; 1; sq] -> sq_j - 2 q_j.q_i + sq_i.
  - conv biases bv, bt cancel exactly: attention rows sum to 1, so bv
    shifts t by a per-channel constant; constants cancel inside
    BatchNorm (train mode). They are dropped.
  - The softmax normalization commutes with the channel matmul:
    t = wt@(xr_un * rep) = (wt@xr_un) * rep, so the reciprocal-денom
    broadcast is folded into the post-matmul PSUM->SBUF move, off the
    critical path.
  - rstd = exp(-0.5*ln(var+eps)) so the tail reuses the exp table set
    instead of loading the sqrt/rsqrt tables.

Performance structure (vs the all-fp32 baseline):
  - All dense matmuls run at 1 cycle/column: float32r for the
    setup/d2/t GEMMs (tf32-class accuracy), float16 for attn@v.
  - sqrt and exp live in different ACT table sets, so the kernel is
    split into phase A (all 16 sqrt chunks) and phase B (all 16 exp
    chunks) with exactly one table load each; the exp^T@v matmuls
    chase the exps chunk-by-chunk.
  - l2/E matrix is stored once in SBUF as fp16 [P, 16, 2048]
    (64 KB/partition); exp runs in-place with an fp32 row-sum
    accumulator for the softmax denominators.
  - Setup is a 4-block pipeline (x DMA -> q matmul -> copies/squares
    split between ACT and DVE -> sq matmul), with the vT matmuls
    interleaved, so the first sqrt starts ~15us in.
"""
import sys
sys.path.insert(0, '/opt/trn_rl_repo')
import numpy as np

B, C, N = 8, 256, 2048
C4 = C // 4
P = 128
JC = N // P          # 16 j-chunks
NB = N // 512        # 4 column blocks of 512
NCORES = 8
BN_EPS = 1e-5
INV_BN = 1.0 / (B * N)

_CACHE = {}
USE_DVE_POW = False  # walrus rejects pow on DVE (tensor_scalar_valid_ops)


def _build(sim=False):
    import concourse.bass as bass
    import concourse.tile as tile
    from concourse import bacc, mybir
    f32 = mybir.dt.float32
    f32r = mybir.dt.float32r
    f16 = mybir.dt.float16

    nc = bacc.Bacc("TRN2", target_bir_lowering=False, debug=False,
                   num_devices=(1 if sim else NCORES))
    x_d = nc.dram_tensor("x", [2, P, N], f32r, kind="ExternalInput")
    wq_d = nc.dram_tensor("wqT", [P, 2, C4], f32r, kind="ExternalInput")
    wv_d = nc.dram_tensor("wvT", [P, 2, C], f32r, kind="ExternalInput")
    wt_d = nc.dram_tensor("wtT", [P, 2, C], f32r, kind="ExternalInput")
    eye_d = nc.dram_tensor("eyem", [P, P], mybir.dt.uint8, kind="ExternalInput")
    gb_d = nc.dram_tensor("gb", [P, 2, 2], f32, kind="ExternalInput")
    one_d = nc.dram_tensor("onesrow", [1, N], f32r, kind="ExternalInput")
    out_d = nc.dram_tensor("out", [2, P, N], f32r, kind="ExternalOutput")

    AF = mybir.ActivationFunctionType
    OP = mybir.AluOpType

    def r(ap):
        return ap if ap.dtype == f32r else ap.bitcast(f32r)

    def blk(nb):
        return slice(512 * nb, 512 * (nb + 1))

    with tile.TileContext(nc) as tc:
        with tc.tile_pool(name="perm", bufs=1) as perm, \
             tc.tile_pool(name="dram", bufs=1, space="DRAM") as dram:
            # ---- permanent SBUF tiles
            xw = perm.tile([P, 2, N], f32r)
            wq = perm.tile([P, 2, C4], f32r)
            wv = perm.tile([P, 2, C], f32r)
            wt = perm.tile([P, 2, C], f32r)
            eye = perm.tile([P, P], mybir.dt.uint8)
            gb = perm.tile([P, 2, 2], f32)
            zerh = perm.tile([P, P], f16)
            ones64 = perm.tile([C4, 1], f32r)
            At = perm.tile([97, N], f32r)   # rows: 0-63 q, 64 sq, 96 ones
            Bt = perm.tile([97, N], f32r)   # rows: 0-63 -2q, 64 ones, 96 sq
            vT = perm.tile([P, JC, C], f16)
            dencol = perm.tile([P, JC], f32)
            rep = perm.tile([P, N], f32)
            ebig = perm.tile([P, JC, N], f16)   # l2 then E, in place
            xr = perm.tile([P, 2, N], f32r)
            stat = perm.tile([P, 8], f32)

            # zero the dead augmentation rows 64-95 before the ones rows
            # land (rows 64/96 are rewritten below; DVE is idle this early)
            nc.vector.memset(At[64:96, :].bitcast(f32), 0.0)
            nc.vector.memset(Bt[64:96, :].bitcast(f32), 0.0)
            # DMA order = need order: x block 0 and wq first, then the
            # ones rows (needed at the first d2), the fat/late ones after.
            def xsrc(nb):
                # x_d is [2, P, N]; iterate (p, o, c) to match xw's layout
                return bass.AP(tensor=x_d, offset=512 * nb,
                               ap=[[N, P], [P * N, 2], [1, 512]])

            nc.sync.dma_start(xw[:, :, blk(0)], xsrc(0))
            nc.sync.dma_start(wq[:], wq_d.ap())
            for nb in range(1, NB):
                nc.sync.dma_start(xw[:, :, blk(nb)], xsrc(nb))
            nc.sync.dma_start(At[96:97, :], one_d.ap())
            nc.sync.dma_start(Bt[64:65, :], one_d.ap())
            nc.sync.dma_start(wv[:], wv_d.ap())
            nc.sync.dma_start(eye[:], eye_d.ap())
            nc.sync.dma_start(wt[:], wt_d.ap())
            nc.sync.dma_start(gb[:], gb_d.ap())

            nc.vector.memset(ones64[:].bitcast(f32), 1.0)
            # force the initial ACT table load to be the sqrt set (which
            # also contains square/copy/relu) before any other activation
            nc.scalar.activation(stat[0:C4, 7:8], ones64[:], AF.Sqrt)
            nc.vector.memset(zerh[:], 0.0)

            # ---- setup pipeline over the 4 column blocks:
            #   q mm -> At copy (DVE) -> q^2 (ACT Square, scratch in xr)
            #   -> sq mm -> At64 (ACT) / Bt96 (DVE); -2q (DVE) hangs off
            #   the At copy only. The vT matmuls come after, overlapping
            #   phase A via their own small PSUM pool.
            with tc.tile_pool(name="ps_set", bufs=2, space="PSUM") as pss:
                for nb in range(NB):
                    pq = pss.tile([C4, 512], f32, tag="pq")
                    nc.tensor.matmul(pq[:], lhsT=r(wq[:, 0, :]),
                                     rhs=r(xw[:, 0, blk(nb)]),
                                     start=True, stop=False)
                    nc.tensor.matmul(pq[:], lhsT=r(wq[:, 1, :]),
                                     rhs=r(xw[:, 1, blk(nb)]),
                                     start=False, stop=True)
                    nc.vector.tensor_copy(out=At[0:C4, blk(nb)], in_=pq[:])
                    # q^2 scratch in xr (dead until the tail)
                    nc.scalar.activation(xr[0:C4, 0, blk(nb)],
                                         At[0:C4, blk(nb)], AF.Square)
                    psq = pss.tile([1, 512], f32, tag="psq")
                    nc.tensor.matmul(psq[:], lhsT=r(ones64[:]),
                                     rhs=r(xr[0:C4, 0, blk(nb)]),
                                     start=True, stop=True)
                    nc.scalar.activation(At[C4:C4 + 1, blk(nb)], psq[:], AF.Copy)
                    nc.vector.tensor_copy(out=Bt[96:97, blk(nb)], in_=psq[:])
                    nc.vector.tensor_scalar(out=Bt[0:C4, blk(nb)],
                                            in0=At[0:C4, blk(nb)],
                                            scalar1=-2.0, scalar2=0.0,
                                            op0=OP.mult, op1=OP.add)

            # ---- phase A: d2 -> sqrt -> l2 (fp16) + diagonal zero.
            # pd2 is 3x [P, 1024] (6 banks) so the vT pool (2 banks) can
            # coexist and the vT matmuls/copies fill phase A's PE/DVE slack.
            with tc.tile_pool(name="ps_v", bufs=2, space="PSUM") as psv, \
                 tc.tile_pool(name="ps_d2", bufs=3, space="PSUM") as psd:
                for a in range(JC):
                    for h in range(2):
                        cols = slice(1024 * h, 1024 * (h + 1))
                        pd2 = psd.tile([P, 1024], f32, tag="d2")
                        for q2 in range(2):
                            nc.tensor.matmul(pd2[:, 512 * q2:512 * (q2 + 1)],
                                             lhsT=r(At[:, a * P:(a + 1) * P]),
                                             rhs=r(Bt[:, 1024 * h + 512 * q2:
                                                      1024 * h + 512 * (q2 + 1)]),
                                             start=True, stop=True)
                        if USE_DVE_POW and h == 1 and a % 2 == 1:
                            # offload 1/4 of the sqrt work to the idle
                            # vector engine (x**0.5 via the pow ALU op)
                            nc.vector.tensor_scalar(
                                out=ebig[:, a, cols], in0=pd2[:],
                                scalar1=0.5, scalar2=0.0,
                                op0=OP.pow, op1=OP.bypass)
                        else:
                            nc.scalar.activation(ebig[:, a, cols], pd2[:], AF.Sqrt)
                        if a // 8 == h:
                            # exact-zero the diagonal block (kills NaN
                            # from sqrt of tiny negatives)
                            nc.vector.copy_predicated(
                                out=ebig[:, a, a * P:(a + 1) * P],
                                mask=eye[:], data=zerh[:])
                for jc0 in range(0, JC, 2):
                    # two vT chunks per PSUM bank: halves the pool-recycle
                    # stalls on the in-order PE queue
                    pv = psv.tile([P, 2, C], f32, tag="pv")
                    for dj in range(2):
                        jc = jc0 + dj
                        nc.tensor.matmul(pv[:, dj, :],
                                         lhsT=r(xw[:, 0, jc * P:(jc + 1) * P]),
                                         rhs=r(wv[:, 0, :]), start=True, stop=False)
                        nc.tensor.matmul(pv[:, dj, :],
                                         lhsT=r(xw[:, 1, jc * P:(jc + 1) * P]),
                                         rhs=r(wv[:, 1, :]), start=False, stop=True)
                    nc.vector.tensor_copy(out=vT[:, jc0:jc0 + 2, :], in_=pv[:])

            # ---- phase B: exp (+den accum) chased by attn@v matmuls
            psav_cm = tc.tile_pool(name="ps_av", bufs=1, space="PSUM")
            psav = psav_cm.__enter__()
            pav = [psav.tile([P, 512], f32, tag=f"av{i}", name=f"pav{i}")
                   for i in range(8)]
            for a in range(JC):
                Pst = ebig[:, a, :]
                nc.scalar.activation(Pst, Pst, AF.Exp, scale=-1.0,
                                     accum_out=dencol[:, a:a + 1])
                for oc in range(2):
                    for ib in range(NB):
                        nc.tensor.matmul(
                            pav[oc * NB + ib][:],
                            lhsT=vT[:, a, oc * P:(oc + 1) * P],
                            rhs=Pst[:, ib * 512:(ib + 1) * 512],
                            start=(a == 0), stop=(a == JC - 1))
                if a % 4 == 3:
                    # denominators -> reciprocal -> broadcast row; four
                    # quarters (one per tail column block) so only the
                    # last quarter's round trip trails the final exp.
                    h = a // 4
                    rden = perm.tile([P, 4], f32, tag=f"rden{h}",
                                     name=f"rden{h}")
                    nc.vector.reciprocal(rden[:], dencol[:, 4 * h:4 * (h + 1)])
                    dden = dram.tile([512], f32, tag=f"dden{h}",
                                     name=f"dden{h}")
                    nc.sync.dma_start(dden.rearrange("(a r) -> r a", r=P), rden[:])
                    bsrc = bass.AP(tensor=dden.tensor, offset=dden.offset,
                                   ap=[[0, P], [1, 512]])
                    nc.sync.dma_start(rep[:, 512 * h:512 * (h + 1)], bsrc)

            # ---- xr_un = pav (move to SBUF), ib-major so t can chase;
            # oc=0 rides ACT (Copy is in every table set), oc=1 rides DVE.
            for ib in range(NB):
                nc.scalar.activation(xr[:, 0, blk(ib)], pav[ib][:], AF.Copy)
                nc.vector.tensor_copy(out=xr[:, 1, blk(ib)], in_=pav[NB + ib][:])

            psav_cm.__exit__(None, None, None)
            # ---- t = (wtT . xr_un) * rep, written back into xr in place;
            # the rep multiply carries the s1 accumulation.
            with tc.tile_pool(name="ps_t", bufs=2, space="PSUM") as pst:
                s1c = [perm.tile([P, NB], f32, name=f"s1c{o}", tag=f"s1c{o}")
                       for o in range(2)]
                s2c = [perm.tile([P, NB], f32, name=f"s2c{o}", tag=f"s2c{o}")
                       for o in range(2)]
                for nb in range(NB):
                    ptl = []
                    for oc2 in range(2):
                        pt = pst.tile([P, 512], f32, tag=f"t{oc2}", name=f"pt{oc2}")
                        nc.tensor.matmul(pt[:], lhsT=r(wt[:, 0, oc2 * P:(oc2 + 1) * P]),
                                         rhs=r(xr[:, 0, blk(nb)]),
                                         start=True, stop=False)
                        nc.tensor.matmul(pt[:], lhsT=r(wt[:, 1, oc2 * P:(oc2 + 1) * P]),
                                         rhs=r(xr[:, 1, blk(nb)]),
                                         start=False, stop=True)
                        ptl.append(pt)
                    for oc2 in range(2):
                        nc.vector.scalar_tensor_tensor(
                            out=xr[:, oc2, blk(nb)], in0=ptl[oc2][:],
                            scalar=1.0, in1=rep[:, blk(nb)],
                            op0=OP.mult, op1=OP.mult,
                            accum_out=s1c[oc2][:, nb:nb + 1])
                        # s2 partial (ACT Square, per block, chases the STT)
                        nc.scalar.activation(
                            out=ebig[:, oc2, blk(nb)],
                            in_=xr[:, oc2, blk(nb)], func=AF.Square,
                            accum_out=s2c[oc2][:, nb:nb + 1])

                # ---- stats: one free-dim reduce per quantity -> stat[:, 0:4]
                for oc2 in range(2):
                    nc.vector.tensor_reduce(out=stat[:, oc2:oc2 + 1],
                                            in_=s1c[oc2][:],
                                            axis=mybir.AxisListType.X, op=OP.add)
                    nc.vector.tensor_reduce(out=stat[:, 2 + oc2:3 + oc2],
                                            in_=s2c[oc2][:],
                                            axis=mybir.AxisListType.X, op=OP.add)

                # ---- AllReduce stats across 8 cores
                cin = dram.tile([P, 4], f32)
                cout = dram.tile([P, 4], f32, addr_space="Shared")
                nc.sync.dma_start(cin[:], stat[:, 0:4])
                if sim:
                    nc.sync.dma_start(cout[:], cin[:])
                else:
                    nc.gpsimd.collective_compute(
                        "AllReduce", OP.add,
                        replica_groups=[list(range(NCORES))],
                        ins=[cin.opt()], outs=[cout.opt()])
                sg = perm.tile([P, 4], f32)
                nc.sync.dma_start(sg[:], cout[:])

                # ---- BN affine params per channel half
                epst = perm.tile([P, 1], f32)
                nc.vector.memset(epst[:], BN_EPS)
                Ak = [perm.tile([P, 1], f32, name=f"Ak{o}", tag=f"Ak{o}") for o in range(2)]
                Bk = [perm.tile([P, 1], f32, name=f"Bk{o}", tag=f"Bk{o}") for o in range(2)]
                mean = perm.tile([P, 2], f32)
                var = perm.tile([P, 2], f32)
                for oc2 in range(2):
                    nc.vector.tensor_scalar(out=mean[:, oc2:oc2 + 1],
                                            in0=sg[:, oc2:oc2 + 1],
                                            scalar1=INV_BN, scalar2=0.0,
                                            op0=OP.mult, op1=OP.add)
                    # var = s2/BN - mean^2
                    nc.vector.scalar_tensor_tensor(
                        out=var[:, oc2:oc2 + 1], in0=mean[:, oc2:oc2 + 1],
                        scalar=1.0, in1=mean[:, oc2:oc2 + 1],
                        op0=OP.mult, op1=OP.mult)
                    nc.vector.scalar_tensor_tensor(
                        out=var[:, oc2:oc2 + 1], in0=sg[:, 2 + oc2:3 + oc2],
                        scalar=INV_BN, in1=var[:, oc2:oc2 + 1],
                        op0=OP.mult, op1=OP.subtract)
                    # rstd = 1/sqrt(var+eps): Sqrt's table set also holds
                    # Relu, so the tail needs exactly one set switch
                    nc.scalar.activation(var[:, oc2:oc2 + 1], var[:, oc2:oc2 + 1],
                                         AF.Sqrt, bias=epst[:])
                    nc.vector.reciprocal(var[:, oc2:oc2 + 1], var[:, oc2:oc2 + 1])
                    # Ak = gamma*rstd ; Bk = beta - mean*Ak
                    nc.vector.tensor_tensor(out=Ak[oc2][:], in0=gb[:, oc2, 0:1],
                                            in1=var[:, oc2:oc2 + 1], op=OP.mult)
                    nc.vector.tensor_tensor(out=Bk[oc2][:], in0=mean[:, oc2:oc2 + 1],
                                            in1=Ak[oc2][:], op=OP.mult)
                    nc.vector.tensor_tensor(out=Bk[oc2][:], in0=gb[:, oc2, 1:2],
                                            in1=Bk[oc2][:], op=OP.subtract)

                # ---- out = x + relu(Ak*t + Bk), 4 pipelined half-row
                # units; the residual add runs in place over xw.
                for h in range(2):
                    for oc2 in range(2):
                        cols = slice(1024 * h, 1024 * (h + 1))
                        u = ebig[:, 4 + 4 * oc2 + 2 * h, 0:2048].bitcast(f32r)
                        nc.scalar.activation(u, xr[:, oc2, cols], AF.Relu,
                                             scale=Ak[oc2][:], bias=Bk[oc2][:])
                        nc.vector.tensor_tensor(out=xw[:, oc2, cols],
                                                in0=xw[:, oc2, cols],
                                                in1=u, op=OP.add)
                        nc.sync.dma_start(out_d.ap()[oc2, :, cols],
                                          xw[:, oc2, cols])

    nc.compile()
    return nc


def _get_nc():
    if "nc" not in _CACHE:
        _CACHE["nc"] = _build()
    return _CACHE["nc"]


def kernel(x, wq, wv, bv, wt, bt, gamma, beta):
    from concourse.bass_utils import run_bass_kernel_spmd

    x = np.ascontiguousarray(np.asarray(x, dtype=np.float32))
    # host-side weight re-layouts (tiny)
    wqT = np.ascontiguousarray(
        np.asarray(wq, np.float32).T.reshape(2, P, C4).transpose(1, 0, 2))
    wvT = np.ascontiguousarray(
        np.asarray(wv, np.float32).T.reshape(2, P, C).transpose(1, 0, 2))
    wtT = np.ascontiguousarray(
        np.asarray(wt, np.float32).T.reshape(2, P, C).transpose(1, 0, 2))
    eyem = np.eye(P, dtype=np.uint8)
    gbh = np.stack([np.asarray(gamma, np.float32).reshape(2, P).T,
                    np.asarray(beta, np.float32).reshape(2, P).T],
                   axis=2).astype(np.float32)  # [P, 2, 2]
    gbh = np.ascontiguousarray(gbh)
    onesr = np.ones((1, N), dtype=np.float32)

    nc = _get_nc()
    in_maps = []
    for b in range(NCORES):
        xb = x[b].reshape(2, P, N)  # contiguous view, no copy
        in_maps.append({"x": xb, "wqT": wqT, "wvT": wvT,
                        "wtT": wtT, "eyem": eyem, "gb": gbh,
                        "onesrow": onesr})
    _CACHE["last_in_maps"] = in_maps
    res = run_bass_kernel_spmd(nc, in_maps, core_ids=list(range(NCORES)))
    _CACHE["last_res"] = res
    out = np.empty((B, C, N), dtype=np.float32)
    for b in range(NCORES):
        out[b] = res.results[b]["out"].reshape(C, N)
    return out
